# revision 24
# baseline (speedup 1.0000x reference)
"""Trainium2 Bass kernel for gnn_message_passing (nn_CMP_67181878444960).

Strategy (8-core SPMD, no collectives):
  - Host converts the edge list into two dense [V, V] count matrices
    (pos / neg).  pooled = A @ feats is then a dense matmul: each core
    computes the pooled features for its 128 nodes by streaming the full
    feats matrix [1024, 16384] through the PE (f32r, K-tiled by 128),
    spilling pooled to a DRAM scratch tensor.
  - The conv encoder is embarrassingly parallel over nodes: each core
    runs 2 residual blocks + final conv/instance-norm/relu for its 128
    nodes.  Convs are 9 shift-tap matmuls (contraction over channels on
    partitions); two nodes are packed per matmul via block-diagonal
    weights (K=96, M=96).  Boundary zero-padding is handled by clipping
    each tap's output window (PSUM has_written gives write-then-
    accumulate semantics).
"""

import functools
import sys

import numpy as np

for _p in ("/opt/trn_rl_repo",):
    if _p not in sys.path:
        sys.path.insert(0, _p)

import concourse.tile as tile  # noqa: E402
from concourse import bacc, bass_utils, mybir  # noqa: E402
from concourse.tile_rust import add_dep_helper  # noqa: E402

F32 = mybir.dt.float32
F32R = mybir.dt.float32r
BF16 = mybir.dt.bfloat16
AF = mybir.ActivationFunctionType

V, C, H = 1024, 16, 32
SP = H * H            # 1024 spatial
PW = H + 2            # padded row width (zero border)
PSP = PW * PW         # padded spatial per channel
CHW = C * SP          # 16384
C3 = 3 * C            # 48 conv channels
NCORES = 8
NPC = V // NCORES     # 128 nodes per core
EPS = 1e-5

# weight-column layout: 5 layers x 9 taps, then the two residual-conv bias rows
_LAYER_COUT = [C3, C3, C3, C3, C]          # 1a, 1b, 2a, 2b, final
_TAP_OFF = []
_off = 0
for _co in _LAYER_COUT:
    _TAP_OFF.append(_off)
    _off += 9 * 2 * _co
_BIAS1B_OFF = _off
_off += 2 * C3
_BIAS2B_OFF = _off
_off += 2 * C3
_ONES_OFF = _off
_off += 512
WCOLS = _off


def _r32(ap):
    return ap.bitcast(F32R)


def _mi(inst):
    return getattr(inst, "ins", inst)


def _interior(t):
    """AP over the H x H interior of a padded [P, PSP] tile."""
    return t[:].rearrange("p (r c) -> p r c", c=PW)[:, 1:H + 1, 1:H + 1]


def _zero_border(nc, t):
    """Zero the 1-px border of a padded tile; returns the memset insts."""
    tr = t[:].rearrange("p (r c) -> p r c", c=PW)
    return [
        nc.vector.memset(tr[:, 0:1, :], 0.0),          # top row
        nc.vector.memset(tr[:, PW - 1:PW, :], 0.0),    # bottom row
        nc.vector.memset(tr[:, 1:PW - 1, 0:1], 0.0),   # left col
        nc.vector.memset(tr[:, 1:PW - 1, PW - 1:PW], 0.0),  # right col
    ]


class _SlotGuard:
    """Explicitly order each pool slot's new first-writer after the previous
    occupant's last accessor (belt-and-braces against mis-synced reuse)."""

    def __init__(self):
        self.state = {}

    def begin(self, tag, bufs, writer_insts):
        idx, hist = self.state.setdefault(tag, [0, {}])
        prev = hist.get(idx % bufs)
        if prev is not None:
            for w in writer_insts:
                add_dep_helper(_mi(w), _mi(prev), True, "slot-reuse guard")

    def end(self, tag, bufs, last_inst):
        st = self.state.setdefault(tag, [0, {}])
        st[1][st[0] % bufs] = last_inst
        st[0] += 1


def build_kernel(tc, aps, npc, v):
    """Emit the per-core program. aps: dict of dram APs."""
    nc = tc.nc
    kt = v // 128            # K-tiles for pooling
    n_chunk = 512            # pooling column chunk
    nchunks = CHW // n_chunk
    npairs = npc // 2

    feats_pool = aps["feats_pool"]
    feats_shard = aps["feats_shard"]
    a_lhsT = aps["a_lhsT"]
    wconv = aps["wconv"]
    biases = aps["biases"]
    out = aps["out"]

    guard = _SlotGuard()
    ctx = {"guard": guard}
    build_kernel._ctx = ctx

    with (
        tc.tile_pool(name="persist", bufs=1) as persist,
        tc.tile_pool(name="psum", bufs=6, space="PSUM") as psum_pool,
        tc.tile_pool(name="psumf", bufs=2, space="PSUM") as psf_pool,
    ):
        # ---- persistent SBUF state ----
        wsb = persist.tile([C3 * 2, WCOLS], BF16, tag="wsb")
        pooled = persist.tile([128, 2 * CHW], BF16, tag="pooled")
        bias_sb = persist.tile([128, 6], F32, tag="bias_sb")
        jt = persist.tile([1, 8], F32, tag="jt")
        ctx["wsb"] = wsb
        ctx["bias_sb"] = bias_sb
        ctx["ones"] = wsb[0:1, _ONES_OFF:_ONES_OFF + 512]

        nc.sync.dma_start(wsb[:], wconv[:, :])
        nc.sync.dma_start(bias_sb[:], biases[:, :])

        # ================= stage 1: pooling =================
        with (
            tc.tile_pool(name="asb", bufs=1) as asb_pool,
            tc.tile_pool(name="fstage", bufs=3) as fstage,
        ):
            a_sb = asb_pool.tile([128, kt * 2 * npc], BF16)
            nc.sync.dma_start(a_sb[:], a_lhsT[:, :])
            for cc in range(nchunks):
                fs = fstage.tile([128, kt * n_chunk], BF16, tag="fs")
                d = nc.sync.dma_start(
                    fs[:], feats_pool[cc * 128:(cc + 1) * 128, :])
                guard.begin("fs", 3, [d])
                last_mm = None
                fs_r = fs[:].rearrange("p (k n) -> p k n", k=kt)
                a_r = a_sb[:].rearrange("p (k m) -> p k m", k=kt)
                for m in range(2):
                    pp = psum_pool.tile([128, n_chunk], F32, tag="ps")
                    for k in range(kt):
                        last_mm = nc.tensor.matmul(
                            pp[:npc, :],
                            a_r[:, k, m * npc:(m + 1) * npc],
                            fs_r[:, k, :],
                            start=(k == 0),
                            stop=(k == kt - 1),
                        )
                    nc.vector.tensor_copy(
                        pooled[:npc, m * CHW + cc * n_chunk:
                               m * CHW + (cc + 1) * n_chunk],
                        pp[:npc, :],
                    )
                guard.end("fs", 3, last_mm)

        # ================= stage 2: conv encoder =================
        with (
            tc.tile_pool(name="stg", bufs=4) as stpool,
            tc.tile_pool(name="xt", bufs=5) as xpool,
            tc.tile_pool(name="ht", bufs=5) as hpool,
            tc.tile_pool(name="ot", bufs=3) as opool,
            tc.tile_pool(name="nrm", bufs=6) as nrm,
        ):
            rider = None
            for p in range(npairs):
                st = stpool.tile([2 * C3, SP], BF16, tag="stg")
                # assemble x = [feats | pooled_pos | pooled_neg] per node
                wrts = []
                for n in range(2):
                    wrts.append(nc.gpsimd.dma_start(
                        st[48 * n:48 * n + 16, :],
                        feats_shard[2 * p + n:2 * p + n + 1, :].rearrange(
                            "o (c s) -> (o c) s", c=C),
                    ))
                    for m in range(2):
                        wrts.append(nc.gpsimd.dma_start(
                            st[48 * n + 16 * (m + 1):48 * n + 16 * (m + 2), :],
                            pooled[2 * p + n:2 * p + n + 1,
                                   m * CHW:(m + 1) * CHW],
                        ))
                guard.begin("stg", 4, wrts)

                x = xpool.tile([2 * C3, PSP], BF16, tag="x")
                bz = _zero_border(nc, x)
                guard.begin("x", 5, bz)
                cpx = nc.vector.tensor_copy(
                    _interior(x),
                    st[:].rearrange("p (r c) -> p r c", c=H),
                )
                guard.end("stg", 4, cpx)

                # residual block 1
                h = hpool.tile([2 * C3, PSP], BF16, tag="h")
                hb = _zero_border(nc, h)
                guard.begin("h", 5, hb)
                hw = _conv(tc, psum_pool, x, h, 0, relu=True, bias_col=0,
                           rider=rider)
                hl = _conv(tc, psum_pool, h, x, 1, resid=True,
                           bias_col=4, tmp_pool=stpool, rider=rider)
                guard.end("h", 5, hl[-1])
                # residual block 2
                h2 = hpool.tile([2 * C3, PSP], BF16, tag="h")
                hb2 = _zero_border(nc, h2)
                guard.begin("h", 5, hb2)
                hw2 = _conv(tc, psum_pool, x, h2, 2, relu=True, bias_col=1,
                            rider=rider)
                hl2 = _conv(tc, psum_pool, h2, x, 3, resid=True,
                            bias_col=5, tmp_pool=stpool, rider=rider)
                guard.end("h", 5, hl2[-1])
                if rider is not None:
                    rider.finish()
                # defer this pair's final conv into the next pair's matmuls
                rider = _FinalRider(tc, psf_pool, opool, nrm, guard,
                                    x, out, p)
            rider.finish()


class _FinalRider:
    """Emits the final 48->16 conv of a pair in PE column-group 3
    (tile_position=(0,96)), interleaved with the next pair's main conv
    matmuls so it runs concurrently in the otherwise-idle array columns."""

    TAPS = [(dy, dx) for dy in (-1, 0, 1) for dx in (-1, 0, 1)]

    def __init__(self, tc, psum_pool, opool, nrm, guard, x, out, p):
        self.tc = tc
        self.guard = guard
        self.x = x
        self.out = out
        self.p = p
        self.i = 0
        self.mainc = 0
        self.last_mm = None
        self.pp = []
        for _ in range(2):
            psf = psum_pool.tile([128, 512], F32, tag="psf")
            self.pp.append(psf)
        self.opool = opool
        self.nrm = nrm

    def _emit_one(self):
        nc = self.tc.nc
        ctx = build_kernel._ctx
        wsb = ctx["wsb"]
        nt, ti = divmod(self.i, 9)
        r0 = nt * 16
        dy, dx = self.TAPS[ti]
        ky, kx = dy + 1, dx + 1
        woff = _TAP_OFF[4] + (ky * 3 + kx) * 2 * C
        xr = self.x[:].rearrange("p (r c) -> p r c", c=PW)
        self.last_mm = nc.tensor.matmul(
            self.pp[nt][96:128, :512],
            wsb[0:2 * C3, woff:woff + 2 * C],
            xr[0:2 * C3, r0 + ky:r0 + ky + 16, kx:kx + H],
            start=(ti == 0), stop=(ti == 8),
            tile_position=(0, 96), skip_group_check=True,
        )
        self.i += 1

    def tick(self):
        self.mainc += 1
        if self.i < 18 and self.mainc % 4 == 0:
            self._emit_one()

    def finish(self):
        while self.i < 18:
            self._emit_one()
        nc = self.tc.nc
        ctx = build_kernel._ctx
        bias_sb = ctx["bias_sb"]
        guard = self.guard
        p = self.p
        ot = self.opool.tile([128, SP], F32, tag="ot")
        ow = []
        for nt in range(2):
            ow.append(nc.scalar.activation(
                ot[96:128, nt * 512:(nt + 1) * 512],
                self.pp[nt][96:128, :],
                AF.Identity, bias=bias_sb[96:128, 2:3],
            ))
        guard.begin("ot", 3, ow)
        guard.end("x", 5, self.last_mm)
        stats = self.nrm.tile([128, 12], F32, tag="stats")
        mv = self.nrm.tile([128, 2], F32, tag="mv")
        sc = self.nrm.tile([128, 3], F32, tag="sc")
        nc.vector.bn_stats(stats[96:128, 0:6], ot[96:128, 0:512])
        nc.vector.bn_stats(stats[96:128, 6:12], ot[96:128, 512:1024])
        nc.vector.bn_aggr(mv[96:128, :], stats[96:128, :])
        nc.scalar.activation(sc[96:128, 0:1], mv[96:128, 1:2], AF.Sqrt,
                             bias=bias_sb[96:128, 3:4])
        nc.vector.reciprocal(sc[96:128, 1:2], sc[96:128, 0:1])
        nc.vector.tensor_scalar(
            sc[96:128, 2:3], mv[96:128, 0:1], sc[96:128, 1:2], -1.0,
            op0=mybir.AluOpType.mult, op1=mybir.AluOpType.mult,
        )
        fin = self.opool.tile([128, SP], F32, tag="fin")
        ap_i = nc.scalar.activation(
            fin[96:128, :], ot[96:128, :], AF.Relu,
            bias=sc[96:128, 2:3], scale=sc[96:128, 1:2],
        )
        guard.begin("fin", 3, [ap_i])
        guard.end("ot", 3, ap_i)
        od = nc.sync.dma_start(self.out[2 * p:2 * p + 2, :], fin[96:128, :])
        guard.end("fin", 3, od)


def _conv(tc, psum_pool, xin, xout, layer, relu=False, resid=False,
          final=False, bias_col=None, bias_off=None, tmp_pool=None,
          rider=None):
    """One 3x3 'SAME' conv for a node pair.

    xin:  [96, 1024] (node, ch) x spatial
    xout: relu  -> write relu(conv+bias) into xout (dense)
          resid -> xout += conv + bias (bias via K=1 ones-matmul)
          final -> copy conv+bias into xout (2*C partitions)
    Returns the per-halftile tail instructions (ACT/DVE).
    """
    nc = tc.nc
    ctx = build_kernel._ctx
    wsb, bias_sb, ones_t = ctx["wsb"], ctx["bias_sb"], ctx["ones"]

    cout = _LAYER_COUT[layer]
    m = 2 * cout
    xr = xin[:].rearrange("p (r c) -> p r c", c=PW)
    if not final:
        outr = xout[:].rearrange("p (r c) -> p r c", c=PW)

    taps = [(dy, dx) for dy in (-1, 0, 1) for dx in (-1, 0, 1)]

    tails = []
    for nt in range(2):
        r0 = nt * 16
        pp = psum_pool.tile([128, 512], F32, tag="ps")
        ppr = pp[:].rearrange("p (r c) -> p r c", c=H)
        first = True
        for i, (dy, dx) in enumerate(taps):
            # out rows r0..r0+16, cols 0..32 read padded window
            ky, kx = dy + 1, dx + 1
            woff = _TAP_OFF[layer] + (ky * 3 + kx) * m
            nc.tensor.matmul(
                pp[:m, :512],
                wsb[0:2 * C3, woff:woff + m],
                xr[0:2 * C3, r0 + ky:r0 + ky + 16, kx:kx + H],
                start=first, stop=(i == len(taps) - 1),
                skip_group_check=True,
            )
            first = False
            if rider is not None:
                rider.tick()
        if relu:
            t = nc.scalar.activation(
                outr[:, 1 + r0:1 + r0 + 16, 1:1 + H], ppr[:m],
                AF.Relu, bias=bias_sb[:m, bias_col:bias_col + 1],
            )
        elif final:
            t = nc.scalar.activation(
                xout[:, nt * 512:(nt + 1) * 512], pp[:m, :],
                AF.Identity, bias=bias_sb[:m, bias_col:bias_col + 1],
            )
        else:  # resid: xout += conv + bias (ACT adds bias, DVE adds x)
            tmp = tmp_pool.tile([2 * C3, 512], BF16, tag="tmp")
            nc.scalar.activation(
                tmp[:m, :], pp[:m, :],
                AF.Identity, bias=bias_sb[:m, bias_col:bias_col + 1],
            )
            t = nc.vector.tensor_add(
                outr[:, 1 + r0:1 + r0 + 16, 1:1 + H],
                outr[:, 1 + r0:1 + r0 + 16, 1:1 + H],
                tmp[:m, :].rearrange("p (r c) -> p r c", c=H),
            )
        tails.append(t)
    return tails


# ======================= host side =======================

def _prep_weights(w_list, b_list):
    """Pack conv weights into the [96, WCOLS] f32 lhsT array."""
    wsb = np.zeros((2 * C3, WCOLS), np.float32)
    for layer, (w, b) in enumerate(zip(w_list, b_list)):
        co = _LAYER_COUT[layer]
        for ky in range(3):
            for kx in range(3):
                lt = np.ascontiguousarray(w[:, :, ky, kx].T)  # [C_in, C_out]
                off = _TAP_OFF[layer] + (ky * 3 + kx) * 2 * co
                wsb[0:C3, off:off + co] = lt
                wsb[C3:2 * C3, off + co:off + 2 * co] = lt
    # residual-conv biases live on partition 0 as K=1 lhsT rows
    wsb[0, _BIAS1B_OFF:_BIAS1B_OFF + 2 * C3] = np.tile(b_list[1], 2)
    wsb[0, _BIAS2B_OFF:_BIAS2B_OFF + 2 * C3] = np.tile(b_list[3], 2)
    wsb[0, _ONES_OFF:_ONES_OFF + 512] = 1.0
    import ml_dtypes
    return wsb.astype(ml_dtypes.bfloat16)


def _prep_biases(b1a, b2a, bf, b1b, b2b):
    bias = np.zeros((128, 6), np.float32)
    bias[0:96, 0] = np.tile(b1a, 2)
    bias[0:96, 1] = np.tile(b2a, 2)
    bias[96:128, 2] = np.tile(bf, 2)
    bias[:, 3] = EPS
    bias[0:96, 4] = np.tile(b1b, 2)
    bias[0:96, 5] = np.tile(b2b, 2)
    return bias


def _build_adjacency(edges, v):
    src, lab, dst = edges[:, 0], edges[:, 1], edges[:, 2]
    a = np.zeros((2, v, v), np.float32)
    for mi, mask in enumerate((lab > 0, lab < 0)):
        s, d = src[mask], dst[mask]
        np.add.at(a[mi], (d, s), 1.0)
        np.add.at(a[mi], (s, d), 1.0)
    return a


@functools.lru_cache(maxsize=2)
def _build_module(npc, v, ncores):
    nc = bacc.Bacc(
        "TRN2", target_bir_lowering=False, debug=False,
        enable_asserts=False, num_devices=ncores,
    )
    aps = {
        "feats_pool": nc.dram_tensor("feats_pool", [(CHW // 512) * 128,
                                     (v // 128) * 512], BF16,
                                     kind="ExternalInput").ap(),
        "feats_shard": nc.dram_tensor("feats_shard", [npc, CHW], BF16,
                                      kind="ExternalInput").ap(),
        "a_lhsT": nc.dram_tensor("a_lhsT", [128, (v // 128) * 2 * npc], BF16,
                                 kind="ExternalInput").ap(),
        "wconv": nc.dram_tensor("wconv", [2 * C3, WCOLS], BF16,
                                kind="ExternalInput").ap(),
        "biases": nc.dram_tensor("biases", [128, 6], F32,
                                 kind="ExternalInput").ap(),
        "out": nc.dram_tensor("out", [npc, CHW], F32,
                              kind="ExternalOutput").ap(),
    }
    with tile.TileContext(nc) as tc:
        build_kernel(tc, aps, npc, v)
    nc.compile()
    return nc


def make_in_maps(feats, edges, w1a, b1a, w1b, b1b, w2a, b2a, w2b, b2b,
                 wf, bf, ncores=NCORES, v=V):
    feats = np.ascontiguousarray(np.asarray(feats, np.float32)).reshape(v, CHW)
    edges = np.asarray(edges)
    npc = v // ncores
    a = _build_adjacency(edges, v)
    wsb = _prep_weights(
        [np.asarray(w) for w in (w1a, w1b, w2a, w2b, wf)],
        [np.asarray(b) for b in (b1a, b1b, b2a, b2b, bf)],
    )
    bias = _prep_biases(np.asarray(b1a), np.asarray(b2a), np.asarray(bf),
                    np.asarray(b1b), np.asarray(b2b))
    in_maps = []
    for i in range(ncores):
        rows = slice(i * npc, (i + 1) * npc)
        a_sel = np.concatenate([a[0, rows], a[1, rows]], axis=0)  # [2*npc, V]
        import ml_dtypes
        kt = v // 128
        nch = CHW // 512
        fp = feats.reshape(kt, 128, nch, 512).transpose(2, 1, 0, 3)
        fp = np.ascontiguousarray(fp).reshape(nch * 128, kt * 512)
        alt = a_sel.T.reshape(kt, 128, 2 * npc).transpose(1, 0, 2)
        alt = np.ascontiguousarray(alt).reshape(128, kt * 2 * npc)
        in_maps.append({
            "feats_pool": fp.astype(ml_dtypes.bfloat16),
            "feats_shard": np.ascontiguousarray(feats[rows]).astype(
                ml_dtypes.bfloat16),
            "a_lhsT": alt.astype(ml_dtypes.bfloat16),
            "wconv": wsb,
            "biases": bias,
        })
    return in_maps


def run(inputs, trace=False):
    in_maps = make_in_maps(**inputs)
    nc = _build_module(NPC, V, NCORES)
    res = bass_utils.run_bass_kernel_spmd(
        nc, in_maps, core_ids=list(range(NCORES)), trace=trace,
    )
    out = np.concatenate(
        [res.results[i]["out"] for i in range(NCORES)], axis=0
    ).reshape(V, C, H, H)
    return out, res


def kernel(**inputs):
    out, _ = run(inputs, trace=False)
    return out


# revision 25
# speedup vs baseline: 1.1743x; 1.1743x over previous
"""Trainium2 Bass kernel for gnn_message_passing (nn_CMP_67181878444960).

Strategy (8-core SPMD, no collectives):
  - Host converts the edge list into two dense [V, V] count matrices
    (pos / neg).  pooled = A @ feats is then a dense matmul: each core
    computes the pooled features for its 128 nodes by streaming the full
    feats matrix [1024, 16384] through the PE (f32r, K-tiled by 128),
    spilling pooled to a DRAM scratch tensor.
  - The conv encoder is embarrassingly parallel over nodes: each core
    runs 2 residual blocks + final conv/instance-norm/relu for its 128
    nodes.  Convs are 9 shift-tap matmuls (contraction over channels on
    partitions); two nodes are packed per matmul via block-diagonal
    weights (K=96, M=96).  Boundary zero-padding is handled by clipping
    each tap's output window (PSUM has_written gives write-then-
    accumulate semantics).
"""

import functools
import sys

import numpy as np

for _p in ("/opt/trn_rl_repo",):
    if _p not in sys.path:
        sys.path.insert(0, _p)

import concourse.tile as tile  # noqa: E402
from concourse import bacc, bass_utils, mybir  # noqa: E402
from concourse.tile_rust import add_dep_helper  # noqa: E402

F32 = mybir.dt.float32
F32R = mybir.dt.float32r
BF16 = mybir.dt.bfloat16
AF = mybir.ActivationFunctionType

V, C, H = 1024, 16, 32
SP = H * H            # 1024 spatial
PW = H + 2            # padded row width (zero border)
PSP = PW * PW         # padded spatial per channel
CHW = C * SP          # 16384
C3 = 3 * C            # 48 conv channels
NCORES = 8
NPC = V // NCORES     # 128 nodes per core
EPS = 1e-5

# weight-column layout: 5 layers x 9 taps, then the two residual-conv bias rows
_LAYER_COUT = [C3, C3, C3, C3, C]          # 1a, 1b, 2a, 2b, final
_TAP_OFF = []
_off = 0
for _co in _LAYER_COUT:
    _TAP_OFF.append(_off)
    _off += 9 * 2 * _co
_BIAS1B_OFF = _off
_off += 2 * C3
_BIAS2B_OFF = _off
_off += 2 * C3
_ONES_OFF = _off
_off += 512
WCOLS = _off


def _r32(ap):
    return ap.bitcast(F32R)


def _mi(inst):
    return getattr(inst, "ins", inst)


def _interior(t):
    """AP over the H x H interior of a padded [P, PSP] tile."""
    return t[:].rearrange("p (r c) -> p r c", c=PW)[:, 1:H + 1, 1:H + 1]


def _zero_border(nc, t):
    """Zero the 1-px border of a padded tile; returns the memset insts."""
    tr = t[:].rearrange("p (r c) -> p r c", c=PW)
    return [
        nc.vector.memset(tr[:, 0:1, :], 0.0),          # top row
        nc.vector.memset(tr[:, PW - 1:PW, :], 0.0),    # bottom row
        nc.vector.memset(tr[:, 1:PW - 1, 0:1], 0.0),   # left col
        nc.vector.memset(tr[:, 1:PW - 1, PW - 1:PW], 0.0),  # right col
    ]


class _SlotGuard:
    """Explicitly order each pool slot's new first-writer after the previous
    occupant's last accessor (belt-and-braces against mis-synced reuse)."""

    def __init__(self):
        self.state = {}

    def begin(self, tag, bufs, writer_insts):
        idx, hist = self.state.setdefault(tag, [0, {}])
        prev = hist.get(idx % bufs)
        if prev is not None:
            for w in writer_insts:
                add_dep_helper(_mi(w), _mi(prev), True, "slot-reuse guard")

    def end(self, tag, bufs, last_inst):
        st = self.state.setdefault(tag, [0, {}])
        st[1][st[0] % bufs] = last_inst
        st[0] += 1


def build_kernel(tc, aps, npc, v):
    """Emit the per-core program. aps: dict of dram APs."""
    nc = tc.nc
    kt = v // 128            # K-tiles for pooling
    n_chunk = 512            # pooling column chunk
    nchunks = CHW // n_chunk
    npairs = npc // 2

    feats_pool = aps["feats_pool"]
    feats_shard = aps["feats_shard"]
    a_lhsT = aps["a_lhsT"]
    wconv = aps["wconv"]
    biases = aps["biases"]
    out = aps["out"]

    guard = _SlotGuard()
    ctx = {"guard": guard}
    build_kernel._ctx = ctx

    with (
        tc.tile_pool(name="persist", bufs=1) as persist,
        tc.tile_pool(name="psum", bufs=8, space="PSUM") as psum_pool,
    ):
        # ---- persistent SBUF state ----
        wsb = persist.tile([C3 * 2, WCOLS], BF16, tag="wsb")
        pooled = persist.tile([128, 2 * CHW], BF16, tag="pooled")
        bias_sb = persist.tile([128, 6], F32, tag="bias_sb")
        jt = persist.tile([1, 8], F32, tag="jt")
        ctx["wsb"] = wsb
        ctx["bias_sb"] = bias_sb
        ctx["ones"] = wsb[0:1, _ONES_OFF:_ONES_OFF + 512]

        nc.sync.dma_start(wsb[:], wconv[:, :])
        nc.sync.dma_start(bias_sb[:], biases[:, :])

        # ================= stage 1: pooling =================
        with (
            tc.tile_pool(name="asb", bufs=1) as asb_pool,
            tc.tile_pool(name="fstage", bufs=4) as fstage,
        ):
            a_sb = asb_pool.tile([128, kt * 2 * npc], BF16)
            nc.sync.dma_start(a_sb[:], a_lhsT[:, :])
            for cc in range(nchunks):
                fs = fstage.tile([128, kt * n_chunk], BF16, tag="fs")
                d = nc.sync.dma_start(
                    fs[:], feats_pool[cc * 128:(cc + 1) * 128, :])
                guard.begin("fs", 4, [d])
                last_mm = None
                fs_r = fs[:].rearrange("p (k n) -> p k n", k=kt)
                a_r = a_sb[:].rearrange("p (k m) -> p k m", k=kt)
                for m in range(2):
                    pp = psum_pool.tile([128, n_chunk], F32, tag="ps")
                    for k in range(kt):
                        last_mm = nc.tensor.matmul(
                            pp[:npc, :],
                            a_r[:, k, m * npc:(m + 1) * npc],
                            fs_r[:, k, :],
                            start=(k == 0),
                            stop=(k == kt - 1),
                        )
                    nc.vector.tensor_copy(
                        pooled[:npc, m * CHW + cc * n_chunk:
                               m * CHW + (cc + 1) * n_chunk],
                        pp[:npc, :],
                    )
                guard.end("fs", 4, last_mm)

        # ================= stage 2: conv encoder =================
        with (
            tc.tile_pool(name="stg", bufs=4) as stpool,
            tc.tile_pool(name="xt", bufs=5) as xpool,
            tc.tile_pool(name="ht", bufs=5) as hpool,
            tc.tile_pool(name="ot", bufs=3) as opool,
            tc.tile_pool(name="nrm", bufs=6) as nrm,
        ):
            for p in range(npairs):
                st = stpool.tile([2 * C3, SP], BF16, tag="stg")
                # assemble x = [feats | pooled_pos | pooled_neg] per node
                wrts = []
                for n in range(2):
                    wrts.append(nc.gpsimd.dma_start(
                        st[48 * n:48 * n + 16, :],
                        feats_shard[2 * p + n:2 * p + n + 1, :].rearrange(
                            "o (c s) -> (o c) s", c=C),
                    ))
                    for m in range(2):
                        wrts.append(nc.gpsimd.dma_start(
                            st[48 * n + 16 * (m + 1):48 * n + 16 * (m + 2), :],
                            pooled[2 * p + n:2 * p + n + 1,
                                   m * CHW:(m + 1) * CHW],
                        ))
                guard.begin("stg", 4, wrts)

                x = xpool.tile([2 * C3, PSP], BF16, tag="x")
                bz = _zero_border(nc, x)
                guard.begin("x", 5, bz)
                cpx = nc.vector.tensor_copy(
                    _interior(x),
                    st[:].rearrange("p (r c) -> p r c", c=H),
                )
                guard.end("stg", 4, cpx)

                # residual block 1
                h = hpool.tile([2 * C3, PSP], BF16, tag="h")
                hb = _zero_border(nc, h)
                guard.begin("h", 5, hb)
                hw = _conv(tc, psum_pool, x, h, 0, relu=True, bias_col=0)
                hl = _conv(tc, psum_pool, h, x, 1, resid=True,
                           bias_col=4, tmp_pool=stpool)
                guard.end("h", 5, hl[-1])
                # residual block 2
                h2 = hpool.tile([2 * C3, PSP], BF16, tag="h")
                hb2 = _zero_border(nc, h2)
                guard.begin("h", 5, hb2)
                hw2 = _conv(tc, psum_pool, x, h2, 2, relu=True, bias_col=1)
                hl2 = _conv(tc, psum_pool, h2, x, 3, resid=True,
                            bias_col=5, tmp_pool=stpool)
                guard.end("h", 5, hl2[-1])
                # final conv + instance norm + relu
                ot = opool.tile([2 * C, SP], F32, tag="ot")
                ow = _conv(tc, psum_pool, x, ot, 4, final=True, bias_col=2)
                guard.begin("ot", 3, ow)
                guard.end("x", 5, ow[-1])

                stats = nrm.tile([2 * C, 12], F32, tag="stats")
                mv = nrm.tile([2 * C, 2], F32, tag="mv")
                sc = nrm.tile([2 * C, 3], F32, tag="sc")
                nc.vector.bn_stats(stats[:, 0:6], ot[:, 0:512])
                nc.vector.bn_stats(stats[:, 6:12], ot[:, 512:1024])
                nc.vector.bn_aggr(mv[:], stats[:])
                # sc0 = sqrt(var+eps); sc1 = 1/sc0; sc2 = -mean/sc0
                nc.scalar.activation(sc[:, 0:1], mv[:, 1:2], AF.Sqrt,
                                     bias=bias_sb[:2 * C, 3:4])
                nc.vector.reciprocal(sc[:, 1:2], sc[:, 0:1])
                nc.vector.tensor_scalar(
                    sc[:, 2:3], mv[:, 0:1], sc[:, 1:2], -1.0,
                    op0=mybir.AluOpType.mult, op1=mybir.AluOpType.mult,
                )
                fin = opool.tile([2 * C, SP], F32, tag="fin")
                ap_i = nc.scalar.activation(
                    fin[:], ot[:], AF.Relu, bias=sc[:, 2:3], scale=sc[:, 1:2]
                )
                guard.begin("fin", 3, [ap_i])
                guard.end("ot", 3, ap_i)
                od = nc.sync.dma_start(out[2 * p:2 * p + 2, :], fin[:])
                guard.end("fin", 3, od)


def _conv(tc, psum_pool, xin, xout, layer, relu=False, resid=False,
          final=False, bias_col=None, bias_off=None, tmp_pool=None):
    """One 3x3 'SAME' conv for a node pair.

    xin:  [96, 1024] (node, ch) x spatial
    xout: relu  -> write relu(conv+bias) into xout (dense)
          resid -> xout += conv + bias (bias via K=1 ones-matmul)
          final -> copy conv+bias into xout (2*C partitions)
    Returns the per-halftile tail instructions (ACT/DVE).
    """
    nc = tc.nc
    ctx = build_kernel._ctx
    wsb, bias_sb, ones_t = ctx["wsb"], ctx["bias_sb"], ctx["ones"]

    cout = _LAYER_COUT[layer]
    m = 2 * cout
    xr = xin[:].rearrange("p (r c) -> p r c", c=PW)
    if not final:
        outr = xout[:].rearrange("p (r c) -> p r c", c=PW)

    taps = [(dy, dx) for dy in (-1, 0, 1) for dx in (-1, 0, 1)]

    tails = []
    for nt in range(2):
        r0 = nt * 16
        pp = psum_pool.tile([128, 512], F32, tag="ps")
        ppr = pp[:].rearrange("p (r c) -> p r c", c=H)
        first = True
        for i, (dy, dx) in enumerate(taps):
            # out rows r0..r0+16, cols 0..32 read padded window
            ky, kx = dy + 1, dx + 1
            woff = _TAP_OFF[layer] + (ky * 3 + kx) * m
            nc.tensor.matmul(
                pp[:m, :512],
                wsb[0:2 * C3, woff:woff + m],
                xr[0:2 * C3, r0 + ky:r0 + ky + 16, kx:kx + H],
                start=first, stop=(i == len(taps) - 1),
                skip_group_check=True,
            )
            first = False
        if relu:
            t = nc.scalar.activation(
                outr[:, 1 + r0:1 + r0 + 16, 1:1 + H], ppr[:m],
                AF.Relu, bias=bias_sb[:m, bias_col:bias_col + 1],
            )
        elif final:
            t = nc.scalar.activation(
                xout[:, nt * 512:(nt + 1) * 512], pp[:m, :],
                AF.Identity, bias=bias_sb[:m, bias_col:bias_col + 1],
            )
        else:  # resid: xout += conv + bias (ACT adds bias, DVE adds x)
            tmp = tmp_pool.tile([2 * C3, 512], BF16, tag="tmp")
            nc.scalar.activation(
                tmp[:m, :], pp[:m, :],
                AF.Identity, bias=bias_sb[:m, bias_col:bias_col + 1],
            )
            t = nc.vector.tensor_add(
                outr[:, 1 + r0:1 + r0 + 16, 1:1 + H],
                outr[:, 1 + r0:1 + r0 + 16, 1:1 + H],
                tmp[:m, :].rearrange("p (r c) -> p r c", c=H),
            )
        tails.append(t)
    return tails


# ======================= host side =======================

def _prep_weights(w_list, b_list):
    """Pack conv weights into the [96, WCOLS] f32 lhsT array."""
    wsb = np.zeros((2 * C3, WCOLS), np.float32)
    for layer, (w, b) in enumerate(zip(w_list, b_list)):
        co = _LAYER_COUT[layer]
        for ky in range(3):
            for kx in range(3):
                lt = np.ascontiguousarray(w[:, :, ky, kx].T)  # [C_in, C_out]
                off = _TAP_OFF[layer] + (ky * 3 + kx) * 2 * co
                wsb[0:C3, off:off + co] = lt
                wsb[C3:2 * C3, off + co:off + 2 * co] = lt
    # residual-conv biases live on partition 0 as K=1 lhsT rows
    wsb[0, _BIAS1B_OFF:_BIAS1B_OFF + 2 * C3] = np.tile(b_list[1], 2)
    wsb[0, _BIAS2B_OFF:_BIAS2B_OFF + 2 * C3] = np.tile(b_list[3], 2)
    wsb[0, _ONES_OFF:_ONES_OFF + 512] = 1.0
    import ml_dtypes
    return wsb.astype(ml_dtypes.bfloat16)


def _prep_biases(b1a, b2a, bf, b1b, b2b):
    bias = np.zeros((128, 6), np.float32)
    bias[0:96, 0] = np.tile(b1a, 2)
    bias[0:96, 1] = np.tile(b2a, 2)
    bias[0:2 * C, 2] = np.tile(bf, 2)
    bias[:, 3] = EPS
    bias[0:96, 4] = np.tile(b1b, 2)
    bias[0:96, 5] = np.tile(b2b, 2)
    return bias


def _build_adjacency(edges, v):
    src, lab, dst = edges[:, 0], edges[:, 1], edges[:, 2]
    a = np.zeros((2, v, v), np.float32)
    for mi, mask in enumerate((lab > 0, lab < 0)):
        s, d = src[mask], dst[mask]
        np.add.at(a[mi], (d, s), 1.0)
        np.add.at(a[mi], (s, d), 1.0)
    return a


@functools.lru_cache(maxsize=2)
def _build_module(npc, v, ncores):
    nc = bacc.Bacc(
        "TRN2", target_bir_lowering=False, debug=False,
        enable_asserts=False, num_devices=ncores,
    )
    aps = {
        "feats_pool": nc.dram_tensor("feats_pool", [(CHW // 512) * 128,
                                     (v // 128) * 512], BF16,
                                     kind="ExternalInput").ap(),
        "feats_shard": nc.dram_tensor("feats_shard", [npc, CHW], BF16,
                                      kind="ExternalInput").ap(),
        "a_lhsT": nc.dram_tensor("a_lhsT", [128, (v // 128) * 2 * npc], BF16,
                                 kind="ExternalInput").ap(),
        "wconv": nc.dram_tensor("wconv", [2 * C3, WCOLS], BF16,
                                kind="ExternalInput").ap(),
        "biases": nc.dram_tensor("biases", [128, 6], F32,
                                 kind="ExternalInput").ap(),
        "out": nc.dram_tensor("out", [npc, CHW], F32,
                              kind="ExternalOutput").ap(),
    }
    with tile.TileContext(nc) as tc:
        build_kernel(tc, aps, npc, v)
    nc.compile()
    return nc


def make_in_maps(feats, edges, w1a, b1a, w1b, b1b, w2a, b2a, w2b, b2b,
                 wf, bf, ncores=NCORES, v=V):
    feats = np.ascontiguousarray(np.asarray(feats, np.float32)).reshape(v, CHW)
    edges = np.asarray(edges)
    npc = v // ncores
    a = _build_adjacency(edges, v)
    wsb = _prep_weights(
        [np.asarray(w) for w in (w1a, w1b, w2a, w2b, wf)],
        [np.asarray(b) for b in (b1a, b1b, b2a, b2b, bf)],
    )
    bias = _prep_biases(np.asarray(b1a), np.asarray(b2a), np.asarray(bf),
                    np.asarray(b1b), np.asarray(b2b))
    in_maps = []
    for i in range(ncores):
        rows = slice(i * npc, (i + 1) * npc)
        a_sel = np.concatenate([a[0, rows], a[1, rows]], axis=0)  # [2*npc, V]
        import ml_dtypes
        kt = v // 128
        nch = CHW // 512
        fp = feats.reshape(kt, 128, nch, 512).transpose(2, 1, 0, 3)
        fp = np.ascontiguousarray(fp).reshape(nch * 128, kt * 512)
        alt = a_sel.T.reshape(kt, 128, 2 * npc).transpose(1, 0, 2)
        alt = np.ascontiguousarray(alt).reshape(128, kt * 2 * npc)
        in_maps.append({
            "feats_pool": fp.astype(ml_dtypes.bfloat16),
            "feats_shard": np.ascontiguousarray(feats[rows]).astype(
                ml_dtypes.bfloat16),
            "a_lhsT": alt.astype(ml_dtypes.bfloat16),
            "wconv": wsb,
            "biases": bias,
        })
    return in_maps


def run(inputs, trace=False):
    in_maps = make_in_maps(**inputs)
    nc = _build_module(NPC, V, NCORES)
    res = bass_utils.run_bass_kernel_spmd(
        nc, in_maps, core_ids=list(range(NCORES)), trace=trace,
    )
    out = np.concatenate(
        [res.results[i]["out"] for i in range(NCORES)], axis=0
    ).reshape(V, C, H, H)
    return out, res


def kernel(**inputs):
    out, _ = run(inputs, trace=False)
    return out
